# revision 12
# baseline (speedup 1.0000x reference)
"""SAGEConv (max aggregation) + log_softmax, distributed over 8 NeuronCores.

Strategy: dst nodes are partitioned across the 8 cores (12500 each).  The
feature table is split into 4 source windows of 25000 rows (int16 gather-
index limit), stored bf16 (256-byte rows — the SWDGE gather is per-index
bound at ~4ns/idx sustained, so halving row bytes costs nothing and halves
SBUF/staging traffic).  Each window table carries a -FLT_MAX filler row
(max identity) and an all-zeros row (empty-aggregation semantics).

Level 1 (per window): each core's dsts are sorted by window in-degree and
packed into bands of 128; band b gets a uniform slot count C = the band's
max window-degree across all cores (SPMD uniformity), giving ~3% padding.
One dma_gather per superblock of bands lands [dst x slot x feat] bf16
tiles; a strided vector reduce_max forms per-dst window partials.

The FINAL dst order equals window 0's order, so window 0's partials are
reduced straight into a resident SBUF tile (no staging or regather).
Windows 1-3 store partials to DRAM staging in window order and are
regathered into resident SBUF tiles in final order.

Scheduling (the critical part — engines execute their queues in program
order, so issue order must match dependency readiness):
  - L1 processes windows [1, 2, 3, 0]; window w's regathers are
    interleaved into the NEXT window's superblock stream with slack, on a
    dedicated SWDGE queue (L1 gathers rotate queues 0-2), so the single
    Pool engine never stalls on a store dependency.
  - Per-band-group combine + matmul + softmax chase window 0's reduce
    wavefront, interleaved into its superblock stream.
  - All group logits go into ONE PSUM tile per group; log_softmax is a
    batched 6-op pass over the group instead of 5 ops per band.

NOTE: transpose-mode dma_gather produced nondeterministically wrong
columns at this scale on hardware (verified by a minimal repro); only
non-transpose gathers are used.  The host un-permutes per-core outputs.
"""

import sys

import numpy as np

sys.path.insert(0, "/opt/trn_rl_repo")

from concourse import bacc, mybir, tile  # noqa: E402
from concourse.masks import make_identity  # noqa: E402

F32 = mybir.dt.float32
BF16 = mybir.dt.bfloat16
I16 = mybir.dt.int16

N_CORES = 8
P = 128
FILL = np.float32(-3.0e38)
S_MAX = 24  # max gather slots per level-1 superblock (256B each)
NB1_MAX = 10  # max bands per level-1 superblock
NB2_MAX = 8  # bands per level-2 group
WAVEFRONT = True  # interleave group computes into the w0 stream
EARLY_RG = True  # interleave L2 regathers into the L1 stream


def build_program(meta, iters=1, mode="full"):
    # mode: "full" | "l1g" (L1 gathers only) | "l1gr" (+reduces) |
    #        "l1" (+stores) | "l2" (L2 only)
    do_l1 = mode in ("full", "l1g", "l1gr", "l1", "l1r2")
    do_l1_reduce = mode in ("full", "l1gr", "l1", "l1r2")
    do_l1_store = mode in ("full", "l1", "l1r2")
    do_rg = mode in ("full", "l2", "l1r2")
    do_l2 = mode in ("full", "l2")
    d = meta["d_in"]
    do = meta["d_out"]
    nb = meta["nb"]
    ndst_pad = meta["ndst_pad"]
    nw = meta["nw"]
    wrows = meta["wrows"]
    nb0a = meta["nb0_active"]
    l1 = meta["l1"]  # per window: list of sb dicts {o0, nslots, bands:[(b,o,C)]}
    idx1_cols = meta["idx1_cols"]  # per window
    l2 = meta["l2"]  # list of (b0, nbands)
    dh = d // 2  # packed f32 columns per feature row

    nc = bacc.Bacc(num_swdge_queues=4, dynamic_dma_scratch_size=32768)
    xw_t = [
        nc.declare_dram_parameter(f"xw{w}", [wrows, d], BF16, isOutput=False)
        for w in range(nw)
    ]
    idx1_t = [
        nc.declare_dram_parameter(
            f"idx1_{w}", [P, max(idx1_cols[w], 1)], I16, isOutput=False
        )
        for w in range(nw)
    ]
    idx2_t = {
        w: nc.declare_dram_parameter(f"idx2_{w}", [P, 8 * nb], I16, isOutput=False)
        for w in range(1, nw)
    }
    xopT_t = nc.declare_dram_parameter("xopT", [d, ndst_pad // 2], F32, isOutput=False)
    wlT_t = nc.declare_dram_parameter("wlT", [d, do], BF16, isOutput=False)
    wrT_t = nc.declare_dram_parameter("wrT", [d, do], BF16, isOutput=False)
    bl_t = nc.declare_dram_parameter("bl", [1, do], BF16, isOutput=False)
    y_t = nc.declare_dram_parameter("y", [ndst_pad, do], F32, isOutput=True)

    import contextlib

    with tile.TileContext(nc) as tc:
        with (
            tc.tile_pool(name="const", bufs=1) as cp,
            tc.tile_pool(name="dram", bufs=1, space="DRAM") as dp,
            tc.tile_pool(name="ps", bufs=4, space="PSUM") as pp,
            tc.tile_pool(name="l1", bufs=7) as wp1,
            tc.tile_pool(name="l2", bufs=3) as wp2,
        ):
            aggw_t = {
                w: dp.tile([ndst_pad + 1, d], BF16, name=f"aggw{w}")
                for w in range(1, nw)
            }

            wl_sb = cp.tile([d, do], BF16)
            nc.sync.dma_start(out=wl_sb[:], in_=wlT_t[:])
            wr_sb = cp.tile([d, do], BF16)
            nc.sync.dma_start(out=wr_sb[:], in_=wrT_t[:])
            bl_sb = cp.tile([1, do], BF16)
            nc.sync.dma_start(out=bl_sb[:], in_=bl_t[:])
            ones_sb = cp.tile([1, P], BF16)
            nc.vector.memset(ones_sb[:], 1.0)
            # resident partials in [dst, feature] layout: window 0 reduced
            # directly (final order), windows 1..nw-1 regathered from staging
            w0agg = cp.tile([P, nb * d], BF16)
            g2res = {
                w: cp.tile([P, nb * d], BF16, name=f"g2res{w}")
                for w in range(1, min(3, nw))
            }
            g3_tiles = {}
            ident = cp.tile([P, P], BF16)
            make_identity(nc, ident[:])
            # FILL row of each staging buffer (read by level-2 for
            # window-degree-0 dsts)
            fill_sb = cp.tile([1, d], BF16)
            nc.vector.memset(fill_sb[:], float(FILL))
            for w in range(1, nw):
                nc.sync.dma_start(
                    out=aggw_t[w][ndst_pad : ndst_pad + 1, :], in_=fill_sb[:]
                )
            if mode == "l2":
                nc.vector.memset(w0agg[:], float(FILL))

            # ---- optional in-NEFF repeat loop for benchmarking ----
            loop_cm = tc.For_i(0, iters, 1) if iters > 1 else contextlib.nullcontext()
            with loop_cm:
                if do_l1 and do_l1_reduce and nb0a < nb:
                    nc.vector.memset(w0agg[:, nb0a * d :], float(FILL))
                # ---- level 1: per-window gather + band reduce ----
                # regather-windows first, window 0 (resident) last.  Window
                # w's L2 regathers are interleaved into the NEXT window's
                # superblock stream with slack so the Pool engine never
                # stalls on a not-yet-satisfied store dependency (every
                # dma_gather is issued by the single Pool engine in program
                # order).
                qn = [0]

                def issue_l1(w, sb):
                    ns = sb["nslots"]
                    bands = sb["bands"]
                    nbs = len(bands)
                    o0 = sb["o0"]
                    ncols = 8 * ns
                    idx_sb = wp1.tile([P, 8 * S_MAX], I16, tag="idx")
                    nc.sync.dma_start(
                        out=idx_sb[:, :ncols],
                        in_=idx1_t[w][:, 8 * o0 : 8 * o0 + ncols],
                    )
                    gt = wp1.tile([P, S_MAX * d], BF16, tag="gt")
                    nc.gpsimd.dma_gather(
                        gt[:, : ns * d].rearrange("p (s f) -> p s f", f=d),
                        xw_t[w][:, :],
                        idx_sb[:, :ncols],
                        128 * ns,
                        128 * ns,
                        d,
                        single_packet=False,
                        queue_num=qn[0] % 3,
                    )
                    qn[0] += 1
                    assert nbs <= NB1_MAX
                    if not do_l1_reduce:
                        return
                    aggsb = None
                    if w > 0:
                        aggsb = wp1.tile([P, NB1_MAX * d], BF16, tag="aggsb")
                    for k, (b, o, c) in enumerate(bands):
                        out_ap = (
                            w0agg[:, b * d : (b + 1) * d]
                            if w == 0
                            else aggsb[:, k * d : (k + 1) * d]
                        )
                        nc.vector.reduce_max(
                            out=out_ap,
                            in_=gt[:, o * d : (o + c) * d].rearrange(
                                "p (c f) -> p f c", f=d
                            ),
                            axis=mybir.AxisListType.X,
                        )
                    if w > 0 and do_l1_store:
                        b0 = bands[0][0]
                        nc.sync.dma_start(
                            out=aggw_t[w][b0 * P : (b0 + nbs) * P, :].rearrange(
                                "(t p) f -> p t f", p=P
                            ),
                            in_=aggsb[:, : nbs * d].rearrange(
                                "p (t f) -> p t f", f=d
                            ),
                        )

                def issue_rg(w, grp, queue):
                    b0, nbs = grp
                    ncols = 8 * nbs
                    idx_sb = wp2.tile([P, 8 * NB2_MAX], I16, tag=f"i2_{w}", bufs=4)
                    nc.sync.dma_start(
                        out=idx_sb[:, :ncols],
                        in_=idx2_t[w][:, 8 * b0 : 8 * b0 + ncols],
                    )
                    if w in g2res:
                        out_ap = g2res[w][:, b0 * d : (b0 + nbs) * d]
                    else:
                        g3 = wp2.tile([P, NB2_MAX * d], BF16, tag="g3", bufs=4)
                        g3_tiles[b0] = g3
                        out_ap = g3[:, : nbs * d]
                    nc.gpsimd.dma_gather(
                        out_ap.rearrange("p (s f) -> p s f", f=d),
                        aggw_t[w][:, :],
                        idx_sb[:, :ncols],
                        128 * nbs,
                        128 * nbs,
                        d,
                        single_packet=False,
                        queue_num=queue,
                    )

                def issue_grp(grp):
                    b0, nbs = grp
                    xop_sb = wp2.tile([P, NB2_MAX * dh], F32, tag="xop")
                    nc.sync.dma_start(
                        out=xop_sb[:, : nbs * dh],
                        in_=xopT_t[:, b0 * dh : (b0 + nbs) * dh],
                    )
                    w0_ap = w0agg[:, b0 * d : (b0 + nbs) * d]
                    rest = [
                        g2res[w][:, b0 * d : (b0 + nbs) * d] for w in sorted(g2res)
                    ]
                    if nw > 3:
                        rest.append(g3_tiles[b0][:, : nbs * d])
                    agg_s = wp2.tile([P, NB2_MAX * d], BF16, tag="agg_s")
                    agg_ap = agg_s[:, : nbs * d]
                    if not rest:
                        nc.vector.tensor_copy(out=agg_ap, in_=w0_ap)
                    else:
                        if len(rest) == 3:
                            t01 = wp2.tile([P, NB2_MAX * d], BF16, tag="t01")
                            nc.vector.tensor_tensor(
                                out=t01[:, : nbs * d],
                                in0=w0_ap,
                                in1=rest[0],
                                op=mybir.AluOpType.max,
                            )
                            t23 = wp2.tile([P, NB2_MAX * d], BF16, tag="t23")
                            nc.vector.tensor_tensor(
                                out=t23[:, : nbs * d],
                                in0=rest[1],
                                in1=rest[2],
                                op=mybir.AluOpType.max,
                            )
                            nc.vector.tensor_tensor(
                                out=agg_ap,
                                in0=t01[:, : nbs * d],
                                in1=t23[:, : nbs * d],
                                op=mybir.AluOpType.max,
                            )
                        else:
                            nc.vector.tensor_tensor(
                                out=agg_ap,
                                in0=w0_ap,
                                in1=rest[0],
                                op=mybir.AluOpType.max,
                            )
                            for r in rest[1:]:
                                nc.vector.tensor_tensor(
                                    out=agg_ap,
                                    in0=agg_ap,
                                    in1=r,
                                    op=mybir.AluOpType.max,
                                )

                    # all bands' logits into ONE PSUM tile, then a batched
                    # 6-op softmax over the whole group (short engine chains)
                    op_all = pp.tile([P, NB2_MAX * do], F32, tag="op")
                    for t in range(nbs):
                        aggT_p = pp.tile([P, d], BF16, tag="tp")
                        nc.tensor.transpose(
                            out=aggT_p[:],
                            in_=agg_s[:, t * d : (t + 1) * d],
                            identity=ident[:],
                        )
                        aggT = wp2.tile([P, d], BF16, tag="aggT")
                        nc.vector.tensor_copy(out=aggT[:], in_=aggT_p[:])
                        nc.tensor.matmul(
                            out=op_all[:, t * do : (t + 1) * do],
                            lhsT=aggT[:],
                            rhs=wl_sb[:],
                            start=True,
                            stop=False,
                        )
                        nc.tensor.matmul(
                            out=op_all[:, t * do : (t + 1) * do],
                            lhsT=xop_sb[:, t * dh : (t + 1) * dh].bitcast(BF16),
                            rhs=wr_sb[:],
                            start=False,
                            stop=False,
                        )
                        nc.tensor.matmul(
                            out=op_all[:, t * do : (t + 1) * do],
                            lhsT=ones_sb[:1, :],
                            rhs=bl_sb[:1, :],
                            start=False,
                            stop=True,
                        )

                    negm = wp2.tile([P, NB2_MAX], F32, tag="negm")
                    nc.vector.reduce_max(
                        out=negm[:, :nbs],
                        in_=op_all[:, : nbs * do].rearrange(
                            "p (t c) -> p t c", c=do
                        ),
                        axis=mybir.AxisListType.X,
                        negate=True,
                    )
                    zc = wp2.tile([P, NB2_MAX * do], F32, tag="zc")
                    nc.vector.tensor_tensor(
                        out=zc[:, : nbs * do].rearrange("p (t c) -> p t c", c=do),
                        in0=op_all[:, : nbs * do].rearrange(
                            "p (t c) -> p t c", c=do
                        ),
                        in1=negm[:, :nbs]
                        .rearrange("p (t o) -> p t o", o=1)
                        .to_broadcast([P, nbs, do]),
                        op=mybir.AluOpType.add,
                    )
                    e = wp2.tile([P, NB2_MAX * do], F32, tag="e")
                    nc.scalar.activation(
                        out=e[:, : nbs * do],
                        in_=zc[:, : nbs * do],
                        func=mybir.ActivationFunctionType.Exp,
                    )
                    s = wp2.tile([P, NB2_MAX], F32, tag="s")
                    nc.vector.reduce_sum(
                        out=s[:, :nbs],
                        in_=e[:, : nbs * do].rearrange("p (t c) -> p t c", c=do),
                        axis=mybir.AxisListType.X,
                    )
                    ls = wp2.tile([P, NB2_MAX], F32, tag="ls")
                    nc.scalar.activation(
                        out=ls[:, :nbs],
                        in_=s[:, :nbs],
                        func=mybir.ActivationFunctionType.Ln,
                    )
                    fin = wp2.tile([P, NB2_MAX * do], F32, tag="fin")
                    nc.vector.tensor_tensor(
                        out=fin[:, : nbs * do].rearrange("p (t c) -> p t c", c=do),
                        in0=zc[:, : nbs * do].rearrange("p (t c) -> p t c", c=do),
                        in1=ls[:, :nbs]
                        .rearrange("p (t o) -> p t o", o=1)
                        .to_broadcast([P, nbs, do]),
                        op=mybir.AluOpType.subtract,
                    )
                    nc.sync.dma_start(
                        out=y_t[b0 * P : (b0 + nbs) * P, :].rearrange(
                            "(t p) c -> p t c", p=P
                        ),
                        in_=fin[:, : nbs * do].rearrange("p (t c) -> p t c", c=do),
                    )

                if do_l1:
                    worder = list(range(1, nw)) + [0]
                    pending = []  # regather thunks awaiting interleave
                    SLACK = 1
                    for w in worder:
                        sbs = l1[w]
                        cur = list(pending)
                        pending = []
                        pi = 0
                        grp_q = list(l2) if (w == 0 and do_l2) else []
                        gq = 0
                        for j, sb in enumerate(sbs):
                            issue_l1(w, sb)
                            if cur and j >= SLACK:
                                rem_sb = max(len(sbs) - 1 - j, 1)
                                want = -((len(cur) - pi) // -rem_sb)
                                for _ in range(want):
                                    if pi < len(cur):
                                        cur[pi]()
                                        pi += 1
                            if w == 0 and do_l2 and WAVEFRONT:
                                # group computes chase the w0 band wavefront
                                # (lag 2 behind the w3 regathers so no
                                # engine-queue stall on not-yet-landed data)
                                covered = sb["bands"][-1][0] + 1
                                while gq < len(grp_q):
                                    b0g, nbsg = grp_q[gq]
                                    if b0g + nbsg > covered:
                                        break
                                    if cur and pi < gq + 2:
                                        break
                                    issue_grp(grp_q[gq])
                                    gq += 1
                        while pi < len(cur):
                            cur[pi]()
                            pi += 1
                        while gq < len(grp_q):
                            issue_grp(grp_q[gq])
                            gq += 1
                        if w > 0 and do_l1_store and do_rg and EARLY_RG:
                            pending = [
                                (lambda w_=w, g_=g: issue_rg(w_, g_, 3))
                                for g in l2
                            ]
                    for t in pending:
                        t()
                    if do_rg and not EARLY_RG:
                        for w in sorted(g2res):
                            for gi, g in enumerate(l2):
                                issue_rg(w, g, gi % 4)
                        if do_l2:
                            for g in l2:
                                issue_grp(g)
                elif do_rg:
                    # l2-only mode: run all regathers standalone
                    for w in sorted(g2res):
                        for gi, g in enumerate(l2):
                            issue_rg(w, g, gi % 4)
                    if do_l2:
                        for g in l2:
                            issue_grp(g)
    nc.compile()
    return nc


def prepare(x, edge_index, W_l, b_l, W_r, n_cores=N_CORES, window_rows=25000):
    import ml_dtypes

    bf16 = ml_dtypes.bfloat16
    x = np.ascontiguousarray(np.asarray(x, dtype=np.float32))
    n, d = x.shape
    do = W_l.shape[0]
    src = np.asarray(edge_index[0], dtype=np.int64)
    dst = np.asarray(edge_index[1], dtype=np.int64)

    # drop duplicate (src,dst) pairs: max-aggregation is idempotent, and
    # every removed edge is one fewer SWDGE descriptor (the bottleneck)
    ekey = dst * np.int64(n) + src
    ekey = np.unique(ekey)
    dst = ekey // n
    src = ekey % n

    npc = (n + n_cores - 1) // n_cores
    nb = (npc + P - 1) // P
    ndst_pad = nb * P
    nw = (n + window_rows - 1) // window_rows
    wrows = window_rows + 2
    w_fill = window_rows  # local index of -FLT_MAX row
    w_zero = window_rows + 1  # local index of zeros row

    total_deg = np.bincount(dst, minlength=n).astype(np.int64)
    x_bf = x.astype(bf16)

    # window data (bf16 packed into f32 pairs) + CSR
    xw = []
    deg_w = []
    srcs_w = []
    ptr_w = []
    for w in range(nw):
        lo, hi = w * window_rows, min((w + 1) * window_rows, n)
        arr = np.zeros((wrows, d), dtype=bf16)
        arr[: hi - lo] = x_bf[lo:hi]
        arr[w_fill] = bf16(FILL)
        # arr[w_zero] stays zeros
        xw.append(arr)
        m = (src >= lo) & (src < hi)
        dw = dst[m]
        sw = src[m] - lo
        dg = np.bincount(dw, minlength=n).astype(np.int64)
        eo = np.argsort(dw, kind="stable")
        deg_w.append(dg)
        srcs_w.append(sw[eo])
        pt = np.zeros(n + 1, dtype=np.int64)
        np.cumsum(dg, out=pt[1:])
        ptr_w.append(pt)

    # per-core slot-indexed orderings
    ids_ext_all = []
    keyW_all = []  # [core][window][slot]
    for c in range(n_cores):
        ids = np.arange(c * npc, min((c + 1) * npc, n))
        ids_ext = np.full(ndst_pad, -1, dtype=np.int64)
        ids_ext[: len(ids)] = ids
        ids_ext_all.append(ids_ext)
        kws = []
        for w in range(nw):
            kw = np.zeros(ndst_pad, dtype=np.int64)
            kw[: len(ids)] = deg_w[w][ids]
            if w == 0:
                # total-degree-0 reals and pads get one ZERO-row slot
                kw[: len(ids)][total_deg[ids] == 0] = 1
                kw[len(ids) :] = 1
            kws.append(kw)
        keyW_all.append(kws)

    orderW = [
        [np.argsort(-keyW_all[c][w], kind="stable") for w in range(nw)]
        for c in range(n_cores)
    ]
    # final order = window 0's order (its partials skip staging + regather)
    orderF = [orderW[c][0] for c in range(n_cores)]

    # global per-band slot counts, level-1
    cs1 = []  # [window][band]
    for w in range(nw):
        cs = []
        for b in range(nb):
            cs.append(
                int(max(keyW_all[c][w][orderW[c][w][b * P]] for c in range(n_cores)))
            )
        cs1.append(cs)
    nb0_active = sum(1 for c in cs1[0] if c > 0)

    # superblock packing (bands with C>0 only)
    l1 = []
    idx1_cols = []
    for w in range(nw):
        sbs = []
        cur = None
        o_glob = 0
        for b in range(nb):
            c = cs1[w][b]
            if c == 0:
                continue
            if (
                cur is None
                or cur["nslots"] + c > S_MAX
                or len(cur["bands"]) >= NB1_MAX
            ):
                cur = {"o0": o_glob, "nslots": 0, "bands": []}
                sbs.append(cur)
            cur["bands"].append((b, cur["nslots"], c))
            cur["nslots"] += c
            o_glob += c
        for sb in sbs:
            bs = [b for b, _, _ in sb["bands"]]
            assert bs == list(range(bs[0], bs[0] + len(bs)))
        l1.append(sbs)
        idx1_cols.append(8 * o_glob)

    # level-2 groups
    l2 = []
    b0 = 0
    while b0 < nb:
        nbs = min(NB2_MAX, nb - b0)
        l2.append((b0, nbs))
        b0 += nbs

    meta = {
        "n": n,
        "d_in": d,
        "d_out": do,
        "npc": npc,
        "nb": nb,
        "ndst_pad": ndst_pad,
        "nw": nw,
        "wrows": wrows,
        "nb0_active": nb0_active,
        "l1": l1,
        "l2": l2,
        "idx1_cols": idx1_cols,
        "orders": orderF,
        "ids_ext": ids_ext_all,
    }

    # ---- build index arrays ----
    def pack16(flat):
        # idx position i -> row i%16, col i//16; the 16-partition block is
        # replicated 8x across the 128 partitions (one copy per GPSIMD core)
        m = len(flat)
        mc = (m + 15) // 16
        fl = np.zeros(mc * 16, dtype=np.int16)
        fl[:m] = flat
        block = np.ascontiguousarray(fl.reshape(mc, 16).T)
        return np.tile(block, (8, 1))

    in_maps = []
    wlT = np.ascontiguousarray(np.asarray(W_l, dtype=np.float32).T).astype(bf16)
    wrT = np.ascontiguousarray(np.asarray(W_r, dtype=np.float32).T).astype(bf16)
    bl = np.asarray(b_l, dtype=np.float32).reshape(1, do).astype(bf16)

    for c in range(n_cores):
        ids_ext = ids_ext_all[c]
        im = {"wlT": wlT, "wrT": wrT, "bl": bl}
        for w in range(nw):
            im[f"xw{w}"] = xw[w]

        # level-1 indices
        for w in range(nw):
            ow = orderW[c][w]
            dw = deg_w[w]
            pt = ptr_w[w]
            sw = srcs_w[w]
            segs = []
            for sb in l1[w]:
                ns = sb["nslots"]
                seg = np.full(128 * ns, w_fill, dtype=np.int64)
                for b, o, cbn in sb["bands"]:
                    slots = ow[b * P : (b + 1) * P]
                    nodes = ids_ext[slots]  # -1 for pads
                    real = nodes >= 0
                    dv = np.where(real, dw[np.maximum(nodes, 0)], 0)
                    base = np.where(real, pt[np.maximum(nodes, 0)], 0)
                    J = np.arange(cbn)[None, :]
                    gi = base[:, None] + np.minimum(J, np.maximum(dv - 1, 0)[:, None])
                    vals = np.where(
                        J < dv[:, None],
                        sw[np.minimum(gi, max(len(sw) - 1, 0))] if len(sw) else 0,
                        w_fill,
                    )
                    if w == 0:
                        zero_slot = (~real) | (
                            real & (total_deg[np.maximum(nodes, 0)] == 0)
                        )
                        vals[zero_slot, 0] = w_zero
                    # position i = (o + j)*128 + p
                    ii = ((o + J) * P + np.arange(P)[:, None]).ravel()
                    seg[ii] = vals.ravel()
                segs.append(seg)
            flat = np.concatenate(segs) if segs else np.zeros(0, dtype=np.int64)
            im[f"idx1_{w}"] = (
                pack16(flat) if len(flat) else np.zeros((P, 1), dtype=np.int16)
            )

        # level-2 indices (windows 1..nw-1): for final position r, the
        # window position of that dst's partial
        for w in range(1, nw):
            ow = orderW[c][w]
            posw = np.empty(ndst_pad, dtype=np.int64)
            posw[ow] = np.arange(ndst_pad)
            nb_active = sum(1 for cc in cs1[w] if cc > 0)
            pw = posw[orderF[c]]  # [ndst_pad] in final order
            pw = np.where(pw < nb_active * P, pw, ndst_pad)  # FILL row
            im[f"idx2_{w}"] = pack16(pw)

        # x_own in final order, transposed, bf16-packed
        oF = orderF[c]
        nodes = ids_ext[oF]
        xop = np.zeros((ndst_pad, d), dtype=bf16)
        valid = nodes >= 0
        xop[valid] = x_bf[nodes[valid]]
        im["xopT"] = (
            np.ascontiguousarray(xop.T).view(np.uint16).view(np.float32)
        )

        in_maps.append(im)

    return in_maps, meta


def assemble(results, meta, n_cores=N_CORES):
    y = np.empty((meta["n"], meta["d_out"]), dtype=np.float32)
    for c in range(n_cores):
        oF = meta["orders"][c]
        nodes = meta["ids_ext"][c][oF]
        valid = nodes >= 0
        y[nodes[valid]] = results[c]["y"][valid]
    return y


def kernel(x, edge_index, W_l, b_l, W_r):
    from concourse.bass_utils import run_bass_kernel_spmd

    in_maps, meta = prepare(x, edge_index, W_l, b_l, W_r)
    nc = build_program(meta)
    res = run_bass_kernel_spmd(nc, in_maps, list(range(N_CORES)))
    return assemble(res.results, meta)


if __name__ == "__main__":
    rng = np.random.default_rng(0)
    n, e, d, do = 4000, 32000, 128, 64
    x = rng.standard_normal((n, d)).astype(np.float32)
    ei = rng.integers(0, n, size=(2, e))
    in_maps, meta = prepare(
        x,
        ei,
        rng.standard_normal((do, d)).astype(np.float32),
        np.zeros(do, np.float32),
        rng.standard_normal((do, d)).astype(np.float32),
        window_rows=1000,
    )
    print("nw:", meta["nw"], "l1 sbs:", [len(s) for s in meta["l1"]])
